# revision 2
# baseline (speedup 1.0000x reference)
"""Single-head attention (B=2, S=2048, D=2048, fp32) on 8 trn2 NeuronCores.

Sequence-parallel bf16 kernel, fine-grained tiles for cross-phase pipelining.
4096 tokens split 512/core; cores 0-3 hold batch 0, cores 4-7 batch 1. Each
core computes K^T/V/Q^T for its tokens, K^T/V all-gathered within each 4-core
group, then scoresT -> exp -> attn@V -> @W_o for its 512 queries against the
full 2048 keys of its batch.

All matmuls bf16 (full PE rate), fp32 PSUM. Scores are computed TRANSPOSED
(scoresT[k,q] = KT.T @ QT) so exp output IS attnT -- no PE transposes.
Softmax denominators via ones-vector matmuls over attnT; 1/sqrt(D) folds into
the exp scale; exp(x-2) keeps headroom. Host prepacks DRAM so all loads are
contiguous 1-2KB partition lines. State tensors are 16x [128,512] tiles and
weight streams 8x [128,2,*] tiles per phase so the Tile framework's per-tile
dependency tracking overlaps DMA, PE, and PSUM-evict across phase boundaries.

per-core phases (each 256 mm of K=128, N=512):
  B: KT_shard[e,t] = W_k^T x^T   -> DRAM, AllGather(x4)
  C: V_shard[t,e]  = x W_v       -> DRAM, AllGather(x4)
  D: QT[e,q]       = W_q^T x^T   -> SBUF bf16
  E: attnT[k,q]    = exp(KTg.T QT * scale - 2)   (+16 exp)
  F: outT[e,q]     = Vg.T attnT  (+64 ones-mm for rowsums)
  G: out[q,d]      = (outT.T W_o) / rowsum
"""
import math
import numpy as np
import ml_dtypes

import concourse.bass as bass
import concourse.mybir as mybir
import concourse.tile as tile
from concourse import bacc
from concourse.bass_utils import run_bass_kernel_spmd

F32 = mybir.dt.float32
BF16 = mybir.dt.bfloat16

D = 2048
B = 2
S = 2048
NCORES = 8
GS = 4
TOK = 512
P = 128
NT = 16

EXP_SCALE = 1.0 / math.sqrt(D)
EXP_BIAS = -2.0


def build_attn(n_iters=1, with_collective=True, ld4=False, skip_reload=False):
    nc = bacc.Bacc("TRN2", target_bir_lowering=False, debug=False, num_devices=NCORES)

    xt = nc.dram_tensor("xt", [P, 8192], BF16, kind="ExternalInput")
    wq = nc.dram_tensor("wq", [P, 32768], BF16, kind="ExternalInput")
    wk = nc.dram_tensor("wk", [P, 32768], BF16, kind="ExternalInput")
    wv = nc.dram_tensor("wv", [P, 32768], BF16, kind="ExternalInput")
    wo = nc.dram_tensor("wo", [P, 32768], BF16, kind="ExternalInput")
    out = nc.dram_tensor("out", [TOK, D], F32, kind="ExternalOutput")

    with tile.TileContext(nc) as tc:
        with (
            tc.tile_pool(name="dram", bufs=1, space="DRAM") as dram,
            tc.tile_pool(name="misc", bufs=1) as misc,
            tc.tile_pool(name="xtp", bufs=NT) as xtp,
            tc.tile_pool(name="qo", bufs=NT) as qo,
            tc.tile_pool(name="atp", bufs=NT) as atp,
            tc.tile_pool(name="stream", bufs=16) as stream,
            tc.tile_pool(name="ev8", bufs=4) as ev8,
            tc.tile_pool(name="evf", bufs=4) as evfp,
            tc.tile_pool(name="ps", bufs=8, space="PSUM") as ps,
        ):
            kt_shard = dram.tile([P, 8192], BF16)
            v_shard = dram.tile([P, 8192], BF16)
            kt_g = dram.tile([GS * P, 8192], BF16)
            v_g = dram.tile([GS * P, 4, D], BF16)

            ones = misc.tile([P, 1], BF16, name="ones")
            d_sb = misc.tile([P, 4], F32, name="dsb")
            recip = misc.tile([P, 4], F32, name="recip")
            ebias = misc.tile([P, 1], F32, name="ebias")

            nc.vector.memset(ones[:], 1.0)
            nc.vector.memset(ebias[:], EXP_BIAS)

            _cp_i = [0]

            def COPY(dst, src):
                if _cp_i[0] % 2 == 0:
                    nc.scalar.copy(dst, src)
                else:
                    nc.vector.tensor_scalar_mul(dst, src, 1.0)
                _cp_i[0] += 1

            _ld_i = [0]
            _ld_engines = ([nc.sync, nc.gpsimd, nc.scalar, nc.vector]
                           if ld4 else [nc.sync, nc.gpsimd])

            def LD(dst, src_ap):
                e = _ld_engines[_ld_i[0] % len(_ld_engines)]
                _ld_i[0] += 1
                e.dma_start(dst, src_ap)

            _stream_cache = {}

            def stream_tiles(key, shape, name, loader, n=8):
                """n stream tiles for one outer iteration of phase `key`; with
                skip_reload, only the first outer iteration loads, rest reuse."""
                if skip_reload and key in _stream_cache:
                    return _stream_cache[key]
                ts = []
                for idx in range(n):
                    t = stream.tile(shape, BF16, tag="st", name=f"{name}{idx}")
                    LD(t[:], loader(idx))
                    ts.append(t)
                if skip_reload:
                    _stream_cache[key] = ts
                return ts

            def proj_eT(w_dram, xt_sb, dest_cb):
                """KT/QT: out[e,t] = sum_d W[d,e] x^T[d,t]; psums per e-128chunk."""
                for eg in range(4):
                    wts = stream_tiles(
                        ("p", w_dram.name), [P, 2, 4, P], "wT",
                        lambda d2: w_dram[:, eg * 8192 + d2 * 1024:
                                          eg * 8192 + (d2 + 1) * 1024])
                    psums = [ps.tile([P, TOK], F32, tag="mm", name=f"pb{j}")
                             for j in range(4)]
                    for d2 in range(8):
                        for i in range(2):
                            dt = d2 * 2 + i
                            for j in range(4):
                                nc.tensor.matmul(
                                    psums[j][:], wts[d2][:, i, j, :],
                                    xt_sb[dt][:],
                                    start=(dt == 0), stop=(dt == NT - 1))
                    for j in range(4):
                        dest_cb(eg * 4 + j, psums[j])

            def b_dest(et, psum):
                ev = ev8.tile([P, TOK], BF16, tag="ev", name="evb")
                COPY(ev[:], psum[:])
                nc.sync.dma_start(kt_shard[:, et * TOK:(et + 1) * TOK], ev[:])

            def phase_c(xt_sb):
                for ec in range(4):
                    wts = stream_tiles(
                        "c", [P, 2, TOK], "wV",
                        lambda d2: wv[:, ec * 8192 + d2 * 1024:
                                      ec * 8192 + (d2 + 1) * 1024])
                    psums = [ps.tile([P, TOK], F32, tag="mm", name=f"pc{t}")
                             for t in range(4)]
                    for d2 in range(8):
                        for i in range(2):
                            dt = d2 * 2 + i
                            for tc_ in range(4):
                                nc.tensor.matmul(
                                    psums[tc_][:],
                                    xt_sb[dt][:, tc_ * P:(tc_ + 1) * P],
                                    wts[d2][:, i, :],
                                    start=(dt == 0), stop=(dt == NT - 1))
                    for tc_ in range(4):
                        ev = ev8.tile([P, TOK], BF16, tag="ev", name="evc")
                        COPY(ev[:], psums[tc_][:])
                        nc.sync.dma_start(
                            v_shard[:, tc_ * D + ec * TOK: tc_ * D + (ec + 1) * TOK],
                            ev[:])

            def phase_e(qt_sb, attnT):
                for kc in range(4):
                    kts = stream_tiles(
                        "e", [P, 2, TOK], "kt",
                        lambda e2: kt_g[kc * P:(kc + 1) * P,
                                        e2 * 1024:(e2 + 1) * 1024])
                    psums = [ps.tile([P, TOK], F32, tag="mm", name=f"pe{k}")
                             for k in range(4)]
                    for e2 in range(8):
                        for i in range(2):
                            et = e2 * 2 + i
                            for kk in range(4):
                                nc.tensor.matmul(
                                    psums[kk][:],
                                    kts[e2][:, i, kk * P:(kk + 1) * P],
                                    qt_sb[et][:],
                                    start=(et == 0), stop=(et == NT - 1))
                    for kk in range(4):
                        gk = kc * 4 + kk
                        nc.scalar.activation(
                            attnT[gk][:], psums[kk][:],
                            mybir.ActivationFunctionType.Exp,
                            bias=ebias[:], scale=EXP_SCALE)

            def phase_f(attnT, outT_cb):
                # softmax denominators d[q] = sum_k attnT[k, q]
                dps = ps.tile([P, TOK], F32, tag="mm", name="dps")
                for qc in range(4):
                    for gk in range(NT):
                        nc.tensor.matmul(
                            dps[:, qc:qc + 1],
                            attnT[gk][:, qc * P:(qc + 1) * P],
                            ones[:, :],
                            start=(gk == 0), stop=(gk == NT - 1))
                nc.scalar.copy(d_sb[:], dps[:, 0:4])
                nc.vector.reciprocal(recip[:], d_sb[:])

                for eg in range(4):
                    vts = stream_tiles(
                        "f", [P, 4, TOK], "vc",
                        lambda s: v_g[s * P:(s + 1) * P, :,
                                      eg * TOK:(eg + 1) * TOK], n=4)
                    psums = [ps.tile([P, TOK], F32, tag="mm", name=f"pf{j}")
                             for j in range(4)]
                    for s in range(4):
                        for tq in range(4):
                            gk = s * 4 + tq
                            for j in range(4):
                                nc.tensor.matmul(
                                    psums[j][:],
                                    vts[s][:, tq, j * P:(j + 1) * P],
                                    attnT[gk][:],
                                    start=(gk == 0), stop=(gk == NT - 1))
                    for j in range(4):
                        outT_cb(eg * 4 + j, psums[j])

            def phase_g(outT):
                for dc in range(4):
                    wts = stream_tiles(
                        "g", [P, 2, TOK], "wo",
                        lambda e2: wo[:, dc * 8192 + e2 * 1024:
                                      dc * 8192 + (e2 + 1) * 1024])
                    psums = [ps.tile([P, TOK], F32, tag="mm", name=f"pg{q}")
                             for q in range(4)]
                    for e2 in range(8):
                        for i in range(2):
                            et = e2 * 2 + i
                            for qc in range(4):
                                nc.tensor.matmul(
                                    psums[qc][:],
                                    outT[et][:, qc * P:(qc + 1) * P],
                                    wts[e2][:, i, :],
                                    start=(et == 0), stop=(et == NT - 1))
                    for qc in range(4):
                        evf = evfp.tile([P, TOK], F32, tag="evf", name="evg")
                        nc.vector.tensor_scalar_mul(evf[:], psums[qc][:],
                                                    recip[:, qc:qc + 1])
                        nc.sync.dma_start(
                            out[qc * P:(qc + 1) * P, dc * TOK:(dc + 1) * TOK],
                            evf[:])

            def whole_body():
                xt_sb = []
                for dt in range(NT):
                    t = xtp.tile([P, TOK], BF16, tag="xt", name=f"xt{dt}")
                    LD(t[:], xt[:, dt * TOK:(dt + 1) * TOK])
                    xt_sb.append(t)

                proj_eT(wk, xt_sb, b_dest)
                if with_collective:
                    nc.gpsimd.collective_compute(
                        "AllGather", mybir.AluOpType.bypass,
                        replica_groups=[[0, 1, 2, 3], [4, 5, 6, 7]],
                        ins=[kt_shard[:].opt()], outs=[kt_g[:].opt()],
                    )
                phase_c(xt_sb)
                if with_collective:
                    nc.gpsimd.collective_compute(
                        "AllGather", mybir.AluOpType.bypass,
                        replica_groups=[[0, 1, 2, 3], [4, 5, 6, 7]],
                        ins=[v_shard[:].opt()], outs=[v_g[:].opt()],
                    )

                qt_sb = [qo.tile([P, TOK], BF16, tag="qo", name=f"qt{i}")
                         for i in range(NT)]

                def d_dest(et, psum):
                    COPY(qt_sb[et][:], psum[:])

                proj_eT(wq, xt_sb, d_dest)

                attnT = [atp.tile([P, TOK], BF16, tag="at", name=f"at{i}")
                         for i in range(NT)]
                phase_e(qt_sb, attnT)

                outT = [qo.tile([P, TOK], BF16, tag="qo", name=f"ot{i}")
                        for i in range(NT)]

                def o_dest(et, psum):
                    COPY(outT[et][:], psum[:])

                phase_f(attnT, o_dest)
                phase_g(outT)

            if n_iters == 1:
                whole_body()
            else:
                with tc.For_i(0, n_iters, 1):
                    whole_body()

    nc.compile()
    return nc


_CACHED = {}


def _get_nc():
    if "nc" not in _CACHED:
        _CACHED["nc"] = build_attn()
    return _CACHED["nc"]


def _bf(a):
    return np.ascontiguousarray(a).astype(ml_dtypes.bfloat16)


def _make_in_maps(inputs):
    x = np.asarray(inputs["x"], np.float32)
    W_q = np.asarray(inputs["W_q"], np.float32)
    W_k = np.asarray(inputs["W_k"], np.float32)
    W_v = np.asarray(inputs["W_v"], np.float32)
    W_o = np.asarray(inputs["W_o"], np.float32)

    # wk/wq pack: [p, eg, dt, em, m] <- W[dt*128+p, eg*512+em*128+m]
    def pack_eT(W):
        a = W.reshape(NT, P, 4, 4, P).transpose(1, 2, 0, 3, 4)
        return _bf(a.reshape(P, 32768))

    # wv pack: [p, ec, dt, n] <- W_v[dt*128+p, ec*512+n]
    wv_p = _bf(W_v.reshape(NT, P, 4, TOK).transpose(1, 2, 0, 3).reshape(P, 32768))
    # wo pack: [p, dc, et, j] <- W_o[et*128+p, dc*512+j]
    wo_p = _bf(W_o.reshape(NT, P, 4, TOK).transpose(1, 2, 0, 3).reshape(P, 32768))

    wq_p = pack_eT(W_q)
    wk_p = pack_eT(W_k)

    toks = x.reshape(B * S, D)

    in_maps = []
    for c in range(NCORES):
        xs = toks[c * TOK:(c + 1) * TOK, :]
        # xt pack: [p, dt, t] <- x^T[dt*128+p, t]
        xt_p = _bf(xs.T.reshape(NT, P, TOK).transpose(1, 0, 2).reshape(P, 8192))
        in_maps.append({
            "xt": xt_p, "wq": wq_p, "wk": wk_p, "wv": wv_p, "wo": wo_p,
        })
    return in_maps


def kernel(x, W_q, W_k, W_v, W_o):
    in_maps = _make_in_maps(dict(x=x, W_q=W_q, W_k=W_k, W_v=W_v, W_o=W_o))
    nc = _get_nc()
    res = run_bass_kernel_spmd(nc, in_maps, core_ids=list(range(NCORES)))
    rows = np.concatenate([res.results[c]["out"] for c in range(NCORES)], axis=0)
    return rows.reshape(B, S, D)
